# revision 22
# baseline (speedup 1.0000x reference)
"""GPT2 block kernel for Trainium2, 8 NeuronCores, zero collectives.

Sharding: core c in 0..7 owns batch b = c//4 and query chunk Q = c%4
(rows [512*Q, 512*Q+512) of that batch). Each core computes K/V for its
batch's full 2048-row sequence (duplicated qkv work buys zero cross-core
communication), runs all 16 heads for its 512 queries, then the
row-parallel aproj/FFN for its rows.

Device layouts (f32):
  - natural tiles:    [128 rows(part), cols(free)]
  - transposed tiles: [128 chan(part), rows(free)]  (for matmul contraction)
  - scores S^T:       [k(part), q(free)] so AV needs no P transpose;
                      softmax denominator comes from a ones-column packed
                      into V' ([k, head, 65] with col 64 == 1.0).
LN gamma/beta are folded into the following matmul weights on the host;
b_v is applied after AV (softmax rows sum to 1, so P @ (V + b) = P@V + b).
"""

import sys

for p in ("/opt/pypackages", "/opt/trn_rl_repo"):
    if p not in sys.path:
        sys.path.insert(0, p)

import numpy as np

N_CORES = 8
T = 2048          # seq len per batch
QC = 512          # query rows per core
C = 1024          # d_model
H = 16            # heads
DH = 64           # head dim
F = 4096          # ffn dim
P = 128           # partitions

_PROGRAM = None


def _build_program():
    import concourse.bass as bass
    import concourse.bacc as bacc
    import concourse.mybir as mybir
    import concourse.tile as tile
    from concourse.masks import make_identity
    from contextlib import ExitStack

    f32 = mybir.dt.float32
    AX = mybir.AxisListType
    ALU = mybir.AluOpType
    ACTF = mybir.ActivationFunctionType

    nc = bacc.Bacc(None, target_bir_lowering=False, debug=False)

    # ---- DRAM I/O ------------------------------------------------------
    xq_d = nc.dram_tensor("xq", [P, 4, C], f32, kind="ExternalInput")
    xk_d = nc.dram_tensor("xk", [P, 16, C], f32, kind="ExternalInput")
    qoff_d = nc.dram_tensor("qoff", [P, 1], f32, kind="ExternalInput")
    wqkv_d = nc.dram_tensor("wqkv", [P, 8, 3 * C], f32, kind="ExternalInput")
    wap_d = nc.dram_tensor("waproj", [P, 8, C], f32, kind="ExternalInput")
    wfc_d = nc.dram_tensor("wfc", [P, 8, F], f32, kind="ExternalInput")
    wmp_d = nc.dram_tensor("wmproj", [P, 32, C], f32, kind="ExternalInput")
    bqk_d = nc.dram_tensor("bqk_pm", [P, 16], f32, kind="ExternalInput")
    bv_d = nc.dram_tensor("bv_pm", [P, 8], f32, kind="ExternalInput")
    bfc_d = nc.dram_tensor("bfc_pm", [P, 32], f32, kind="ExternalInput")
    bap_d = nc.dram_tensor("bap_row", [1, C], f32, kind="ExternalInput")
    bmp_d = nc.dram_tensor("bmp_row", [1, C], f32, kind="ExternalInput")
    out_d = nc.dram_tensor("out", [P, 4, C], f32, kind="ExternalOutput")
    # internal bounce for V' (streamed back per head during attention)
    vp_d = nc.dram_tensor("vp_bounce", [P, 16, H, DH + 1], f32)

    with tile.TileContext(nc) as tc, ExitStack() as top:
        const = top.enter_context(tc.tile_pool(name="const", bufs=1))

        # identity via iota+compare (avoids make_identity's affine_select,
        # whose fill-register trips the walrus verifier in this flow)
        ident = const.tile([P, P], f32, tag="ident", name="ident")
        rowi = const.tile([P, P], f32, tag="rowi", name="rowi")
        nc.gpsimd.iota(rowi[:], pattern=[[0, P]], base=0, channel_multiplier=1,
                       allow_small_or_imprecise_dtypes=True)
        coli = const.tile([P, P], f32, tag="coli", name="coli")
        nc.gpsimd.iota(coli[:], pattern=[[1, P]], base=0, channel_multiplier=0,
                       allow_small_or_imprecise_dtypes=True)
        nc.vector.tensor_tensor(ident[:], rowi[:], coli[:], op=ALU.is_equal)
        ones_row = const.tile([1, P], f32, tag="ones_row", name="ones_row")
        nc.vector.memset(ones_row[:], 1.0)
        bqk_sb = const.tile([P, 16], f32, tag="bqk", name="bqk")
        nc.sync.dma_start(out=bqk_sb[:], in_=bqk_d[:])
        bv_sb = const.tile([P, 8], f32, tag="bv", name="bv")
        nc.sync.dma_start(out=bv_sb[:], in_=bv_d[:])
        bfc_sb = const.tile([P, 32], f32, tag="bfc", name="bfc")
        nc.sync.dma_start(out=bfc_sb[:], in_=bfc_d[:])
        qoff_sb = const.tile([P, 1], f32, tag="qoff", name="qoff")
        nc.sync.dma_start(out=qoff_sb[:], in_=qoff_d[:])
        eps_t = const.tile([P, 1], f32, tag="eps", name="eps")
        nc.vector.memset(eps_t[:], 1e-5)

        # broadcast a [1,C] DRAM row to [128,C] via rank-1 matmul.
        def bcast_row(dram, pool, psp, tag):
            row = pool.tile([1, C], f32, tag=tag + "_r", name=tag + "_r")
            nc.sync.dma_start(out=row[:], in_=dram[:])
            dst = pool.tile([P, C], f32, tag=tag, name=tag)
            for j in range(2):
                ps = psp.tile([P, 512], f32, name="psb", bufs=2)
                nc.tensor.matmul(ps[:], ones_row[0:1, :],
                                 row[0:1, 512 * j:512 * j + 512],
                                 start=True, stop=True)
                nc.scalar.copy(dst[:, 512 * j:512 * j + 512], ps[:])
            return dst

        # ---- LN helper: stats + normalize for one [128, C] tile --------
        # bn_stats/bn_aggr keep the whole chain on DVE (one ACT sqrt), which
        # also avoids TensorScalarPtr sync-wait overflow.
        def layernorm_tile(src, dst, stats_pool, _sqp=None):
            bst = stats_pool.tile([P, 2, 6], f32, tag="bst", name="bst")
            mv = stats_pool.tile([P, 2], f32, tag="mv", name="mv")
            x3 = src.rearrange("p (s f) -> p s f", f=512)
            for sg in range(2):
                nc.vector.bn_stats(out=bst[:, sg, :], in_=x3[:, sg, :])
            nc.vector.bn_aggr(out=mv[:], in_=bst[:])
            mean, var = mv[:, 0:1], mv[:, 1:2]
            nc.scalar.activation(out=var, in_=var, func=ACTF.Sqrt,
                                 bias=eps_t[:, 0:1])
            nc.vector.reciprocal(out=var, in_=var)
            nc.vector.tensor_scalar(dst[:], src[:], mean, var,
                                    op0=ALU.subtract, op1=ALU.mult)

        # long-lived activation pools (stack order matters!)
        x2pool = top.enter_context(tc.tile_pool(name="x2pool", bufs=1))
        x2_t = [x2pool.tile([P, C], f32, tag=f"x2{i}", name=f"x2{i}")
                for i in range(4)]
        mid = ExitStack()
        kqy = mid.enter_context(tc.tile_pool(name="kqy", bufs=1))
        qT = [kqy.tile([P, QC], f32, tag=f"qT{i}", name=f"qT{i}")
              for i in range(8)]
        kT = [kqy.tile([P, T], f32, tag=f"kT{i}", name=f"kT{i}")
              for i in range(8)]
        yT = [kqy.tile([P, QC], f32, tag=f"yT{i}", name=f"yT{i}")
              for i in range(8)]

        # ============ phases 1-3: Q^T, LN1(xk), V', K^T =================
        # Q side first: xqnT (16KB) lives briefly, then xknT (64KB).
        with ExitStack() as phQ:
            xqtp = phQ.enter_context(tc.tile_pool(name="xqtp", bufs=1))
            xqnT = [xqtp.tile([P, QC], f32, tag=f"xqnT{i}", name=f"xqnT{i}")
                    for i in range(8)]
            with ExitStack() as ph:
                raw = ph.enter_context(tc.tile_pool(name="xqraw", bufs=2))
                xknp = ph.enter_context(tc.tile_pool(name="xqn", bufs=2))
                stats = ph.enter_context(tc.tile_pool(name="statsq", bufs=4))
                sqp = ph.enter_context(
                    tc.tile_pool(name="sqpq", bufs=2, space="PSUM"))
                pst = ph.enter_context(
                    tc.tile_pool(name="pstq", bufs=4, space="PSUM"))
                for i in range(4):
                    xr = raw.tile([P, C], f32, tag="xr", name="xr")
                    nc.sync.dma_start(out=xr[:], in_=xq_d[:, i, :])
                    xn = xknp.tile([P, C], f32, tag="xn", name="xn")
                    layernorm_tile(xr, xn, stats, sqp)
                    for ci in range(8):
                        ps = pst.tile([P, P], f32, name="pst")
                        nc.tensor.matmul(ps[:], xn[:, P * ci:P * ci + P],
                                         ident[:], is_transpose=True)
                        nc.scalar.copy(xqnT[ci][:, P * i:P * i + P], ps[:])
            with ExitStack() as ph:
                wqp = ph.enter_context(tc.tile_pool(name="wq", bufs=3))
                psq = ph.enter_context(
                    tc.tile_pool(name="psq", bufs=4, space="PSUM"))
                for dcg in range(2):
                    pss = [psq.tile([P, QC], f32, name="psq")
                           for _ in range(4)]
                    for ci in range(8):
                        w = wqp.tile([P, 512], f32, tag="wq", name="wq")
                        nc.sync.dma_start(
                            out=w[:],
                            in_=wqkv_d[:, ci, 512 * dcg:512 * dcg + 512])
                        for g in range(4):
                            nc.tensor.matmul(pss[g][:], w[:, P * g:P * g + P],
                                             xqnT[ci][:],
                                             start=(ci == 0), stop=(ci == 7))
                    for g in range(4):
                        dc = 4 * dcg + g
                        nc.scalar.activation(qT[dc][:], pss[g][:],
                                             ACTF.Identity,
                                             bias=bqk_sb[:, dc:dc + 1])

        with ExitStack() as phA:
            xtp = phA.enter_context(tc.tile_pool(name="xtp", bufs=1))
            xknT = [xtp.tile([P, T], f32, tag=f"xknT{i}", name=f"xknT{i}")
                    for i in range(8)]
            with ExitStack() as ph:
                raw = ph.enter_context(tc.tile_pool(name="xraw", bufs=2))
                xknp = ph.enter_context(tc.tile_pool(name="xkn", bufs=2))
                stats = ph.enter_context(tc.tile_pool(name="stats", bufs=4))
                sqp = ph.enter_context(
                    tc.tile_pool(name="sqp", bufs=2, space="PSUM"))
                pst = ph.enter_context(
                    tc.tile_pool(name="pst", bufs=4, space="PSUM"))
                for i in range(16):
                    xr = raw.tile([P, C], f32, tag="xr", name="xr")
                    nc.sync.dma_start(out=xr[:], in_=xk_d[:, i, :])
                    xn = xknp.tile([P, C], f32, tag="xn", name="xn")
                    layernorm_tile(xr, xn, stats, sqp)
                    for ci in range(8):
                        ps = pst.tile([P, P], f32, name="pst")
                        nc.tensor.matmul(ps[:], xn[:, P * ci:P * ci + P],
                                         ident[:], is_transpose=True)
                        nc.scalar.copy(xknT[ci][:, P * i:P * i + P], ps[:])

            # ---- phase 3a: V' = xkn @ w_v -> DRAM bounce ---------------
            with ExitStack() as ph:
                wvp = ph.enter_context(tc.tile_pool(name="wv", bufs=3))
                vtmp = ph.enter_context(tc.tile_pool(name="vtmp", bufs=2))
                onesp = ph.enter_context(tc.tile_pool(name="onesp", bufs=1))
                psv = ph.enter_context(
                    tc.tile_pool(name="psv", bufs=8, space="PSUM"))
                onescol = onesp.tile([P, 16, H], f32, tag="onescol",
                                     name="onescol")
                nc.vector.memset(onescol[:], 1.0)
                nc.sync.dma_start(out=vp_d[:, :, :, DH:DH + 1],
                                  in_=onescol[:])
                for ktg in range(4):          # groups of 4 k-tiles
                    pss = [psv.tile([P, 512], f32, name="psv")
                           for _ in range(8)]
                    for ci in range(8):
                        w = wvp.tile([P, C], f32, tag="wv", name="wv")
                        nc.sync.dma_start(out=w[:],
                                          in_=wqkv_d[:, ci, 2 * C:3 * C])
                        for g in range(4):
                            kt = 4 * ktg + g
                            for vc in range(2):
                                nc.tensor.matmul(
                                    pss[2 * g + vc][:],
                                    xknT[ci][:, P * kt:P * kt + P],
                                    w[:, 512 * vc:512 * vc + 512],
                                    start=(ci == 0), stop=(ci == 7))
                    for g in range(4):
                        kt = 4 * ktg + g
                        for vc in range(2):
                            vt = vtmp.tile([P, 512], f32, tag="vt",
                                           name="vt")
                            nc.scalar.copy(vt[:], pss[2 * g + vc][:])
                            nc.sync.dma_start(
                                out=vp_d[:, kt, 8 * vc:8 * vc + 8, 0:DH],
                                in_=vt[:])

            # ---- phase 3b: K^T = w_k^T @ xkn^T + b_k -------------------
            with ExitStack() as ph:
                wkp = ph.enter_context(tc.tile_pool(name="wk", bufs=10))
                psk = ph.enter_context(
                    tc.tile_pool(name="psk", bufs=6, space="PSUM"))
                for dc in range(8):
                    wk_ch = []
                    for ci in range(8):
                        w = wkp.tile([P, P], f32, tag="wk", name="wk")
                        nc.sync.dma_start(
                            out=w[:],
                            in_=wqkv_d[:, ci, C + P * dc:C + P * dc + P])
                        wk_ch.append(w)
                    for kc in range(4):
                        ps = psk.tile([P, 512], f32, name="psk")
                        for ci in range(8):
                            nc.tensor.matmul(
                                ps[:], wk_ch[ci][:],
                                xknT[ci][:, 512 * kc:512 * kc + 512],
                                start=(ci == 0), stop=(ci == 7))
                        nc.scalar.activation(
                            kT[dc][:, 512 * kc:512 * kc + 512], ps[:],
                            ACTF.Identity, bias=bqk_sb[:, 8 + dc:9 + dc])

        # ================= phase 4: attention ===========================
        with ExitStack() as ph:
            mkp = ph.enter_context(tc.tile_pool(name="mkp", bufs=1))
            vstr = ph.enter_context(tc.tile_pool(name="vstr", bufs=2))
            ntp = ph.enter_context(tc.tile_pool(name="nt", bufs=4))
            rcpp = ph.enter_context(tc.tile_pool(name="rcp", bufs=2))
            psS = ph.enter_context(
                tc.tile_pool(name="psS", bufs=3, space="PSUM"))
            psY = ph.enter_context(
                tc.tile_pool(name="psY", bufs=2, space="PSUM"))
            psB = ph.enter_context(
                tc.tile_pool(name="psB", bufs=2, space="PSUM"))

            # causal mask [k(part), kt, q] = 1.0 if (128*kt+p) <= q + qoff
            maskT = mkp.tile([P, 16, QC], f32, tag="maskT", name="maskT")
            qq = mkp.tile([P, QC], f32, tag="iota_q", name="iota_q")
            nc.gpsimd.iota(qq[:], pattern=[[1, QC]], base=0,
                           channel_multiplier=0,
                           allow_small_or_imprecise_dtypes=True)
            ktp = mkp.tile([P, 16], f32, tag="iota_k", name="iota_k")
            nc.gpsimd.iota(ktp[:], pattern=[[P, 16]], base=0,
                           channel_multiplier=1,
                           allow_small_or_imprecise_dtypes=True)
            ktq = mkp.tile([P, 16], f32, tag="ktq", name="ktq")
            nc.vector.tensor_scalar(ktq[:], ktp[:], qoff_sb[:, 0:1], None,
                                    op0=ALU.subtract)
            for kt in range(16):
                nc.vector.tensor_scalar(maskT[:, kt, :], qq[:],
                                        ktq[:, kt:kt + 1], None,
                                        op0=ALU.is_ge)

            # Software-pipelined attention stream: S/exp/mask run LAG pairs
            # ahead of AV so the PE never stalls on ACT/DVE latency.
            # Normalization (1/D) is batched after the loop: one DVE
            # reciprocal + 16 rank-1 broadcast matmuls.
            LAG = 3
            drow = mkp.tile([16, QC], f32, tag="drow", name="drow")
            drcp = mkp.tile([16, QC], f32, tag="drcp", name="drcp")
            pairs = [(h, kt) for h in range(H) for kt in range(16)]
            vp_ts, pys, ntq = {}, {}, []
            for j in range(len(pairs) + LAG):
                if j < len(pairs):
                    h, kt = pairs[j]
                    ti, po = h // 2, DH * (h % 2)
                    if kt == 0:
                        vp_t = vstr.tile([P, 16, DH + 1], f32, tag="vp",
                                         name="vp")
                        nc.sync.dma_start(out=vp_t[:], in_=vp_d[:, :, h, :])
                        vp_ts[h] = vp_t
                        pys[h] = psY.tile([DH + 1, QC], f32, name="psY")
                    ps = psS.tile([P, QC], f32, name="psS", bufs=4)
                    nc.tensor.matmul(ps[:],
                                     kT[ti][po:po + DH, P * kt:P * kt + P],
                                     qT[ti][po:po + DH, :],
                                     start=True, stop=True)
                    nt = ntp.tile([P, QC], f32, tag="nt", name="nt", bufs=6)
                    nc.scalar.activation(nt[:], ps[:], ACTF.Exp, scale=0.125)
                    nc.vector.tensor_tensor(nt[:], nt[:], maskT[:, kt, :],
                                            op=ALU.mult)
                    ntq.append((h, kt, nt))
                if j >= LAG:
                    h2, kt2, nt2 = ntq.pop(0)
                    nc.tensor.matmul(pys[h2][:], vp_ts[h2][:, kt2, :],
                                     nt2[:], start=(kt2 == 0),
                                     stop=(kt2 == 15))
                    if kt2 == 15:
                        ti2, po2 = h2 // 2, DH * (h2 % 2)
                        nc.scalar.copy(yT[ti2][po2:po2 + DH, :],
                                       pys[h2][0:DH, :])
                        r1 = rcpp.tile([1, QC], f32, tag="r1",
                                       name="r1", bufs=3)
                        nc.scalar.copy(r1[:], pys[h2][DH:DH + 1, :])
                        nc.sync.dma_start(out=drow[h2:h2 + 1, :],
                                          in_=r1[:])
            nc.vector.reciprocal(drcp[:], drow[:])
            for h in range(H):
                ti, po = h // 2, DH * (h % 2)
                r0 = rcpp.tile([1, QC], f32, tag="rcp", name="rcp")
                nc.sync.dma_start(out=r0[:], in_=drcp[h:h + 1, :])
                pb = psB.tile([DH, QC], f32, name="psB")
                nc.tensor.matmul(pb[:], ones_row[0:1, 0:DH],
                                 r0[:], start=True, stop=True)
                dst = yT[ti][po:po + DH, :]
                nc.vector.tensor_tensor(dst, dst, pb[:], op=ALU.mult)
                nc.vector.tensor_scalar(
                    dst, dst, bv_sb[po:po + DH, h // 2:h // 2 + 1], None,
                    op0=ALU.add)

        # ================= phase 5: aproj + residual ====================
        with ExitStack() as ph:
            wapp = ph.enter_context(tc.tile_pool(name="wap", bufs=1))
            xqr = ph.enter_context(tc.tile_pool(name="xqr", bufs=1))
            bcp = ph.enter_context(tc.tile_pool(name="bcp", bufs=1))
            psa = ph.enter_context(
                tc.tile_pool(name="psa", bufs=4, space="PSUM"))
            atmp = ph.enter_context(tc.tile_pool(name="atmp", bufs=2))
            bapb = bcast_row(bap_d, bcp, psa, "bapb")
            wap_sb = [wapp.tile([P, C], f32, tag=f"wap{i}", name=f"wap{i}")
                      for i in range(8)]
            for dc in range(8):
                nc.sync.dma_start(out=wap_sb[dc][:], in_=wap_d[:, dc, :])
            xq_t = [xqr.tile([P, C], f32, tag=f"xq{i}", name=f"xq{i}")
                    for i in range(4)]
            for i in range(4):
                nc.sync.dma_start(out=xq_t[i][:], in_=xq_d[:, i, :])
            for qt in range(4):
                for oc in range(2):
                    ps = psa.tile([P, 512], f32, name="psa")
                    for dc in range(8):
                        nc.tensor.matmul(
                            ps[:], yT[dc][:, P * qt:P * qt + P],
                            wap_sb[dc][:, 512 * oc:512 * oc + 512],
                            start=(dc == 0), stop=(dc == 7))
                    tt = atmp.tile([P, 512], f32, tag="tt", name="tt")
                    nc.vector.tensor_tensor(
                        tt[:], ps[:], xq_t[qt][:, 512 * oc:512 * oc + 512],
                        op=ALU.add)
                    nc.vector.tensor_tensor(
                        x2_t[qt][:, 512 * oc:512 * oc + 512], tt[:],
                        bapb[:, 512 * oc:512 * oc + 512], op=ALU.add)

        mid.close()   # free qT/kT/yT (96KB) before the FFN phases

        # ================= phases 6-8: LN2 + FFN ========================
        with ExitStack() as phB:
            h2p = phB.enter_context(tc.tile_pool(name="h2p", bufs=1))
            h2T = [h2p.tile([P, QC], f32, tag=f"h2T{i}", name=f"h2T{i}")
                   for i in range(8)]
            with ExitStack() as ph:
                stats = ph.enter_context(tc.tile_pool(name="stats2", bufs=4))
                h2np = ph.enter_context(tc.tile_pool(name="h2n", bufs=2))
                sqp = ph.enter_context(
                    tc.tile_pool(name="sqp2", bufs=2, space="PSUM"))
                pst = ph.enter_context(
                    tc.tile_pool(name="pst2", bufs=4, space="PSUM"))
                for i in range(4):
                    xn = h2np.tile([P, C], f32, tag="h2n", name="h2n")
                    layernorm_tile(x2_t[i], xn, stats, sqp)
                    for ci in range(8):
                        ps = pst.tile([P, P], f32, name="pst")
                        nc.tensor.matmul(ps[:], xn[:, P * ci:P * ci + P],
                                         ident[:], is_transpose=True)
                        nc.scalar.copy(h2T[ci][:, P * i:P * i + P], ps[:])

            with ExitStack() as ph:
                wfcp = ph.enter_context(tc.tile_pool(name="wfcp", bufs=10))
                wmpp = ph.enter_context(tc.tile_pool(name="wmpp", bufs=6))
                h1p = ph.enter_context(tc.tile_pool(name="h1g", bufs=8))
                oap = ph.enter_context(tc.tile_pool(name="oap", bufs=1))
                bcp = ph.enter_context(tc.tile_pool(name="bcp2", bufs=1))
                psf = ph.enter_context(
                    tc.tile_pool(name="psf", bufs=4, space="PSUM"))
                psm = ph.enter_context(
                    tc.tile_pool(name="psm", bufs=2, space="PSUM"))
                bmpb = bcast_row(bmp_d, bcp, psm, "bmpb")
                out_acc = [oap.tile([P, C], f32, tag=f"oa{i}", name=f"oa{i}")
                           for i in range(4)]
                for qt in range(4):
                    nc.vector.tensor_tensor(out_acc[qt][:], x2_t[qt][:],
                                            bmpb[:], op=ALU.add)
                for fig in range(8):          # groups of 4 f-chunks
                    wfc_ch = []
                    for ci in range(8):
                        w = wfcp.tile([P, 512], f32, tag="wfc", name="wfc")
                        nc.sync.dma_start(
                            out=w[:],
                            in_=wfc_d[:, ci, 512 * fig:512 * fig + 512])
                        wfc_ch.append(w)
                    h1g = []
                    for g in range(4):
                        fi = 4 * fig + g
                        pf = psf.tile([P, QC], f32, name="psf")
                        for ci in range(8):
                            nc.tensor.matmul(
                                pf[:], wfc_ch[ci][:, P * g:P * g + P],
                                h2T[ci][:], start=(ci == 0), stop=(ci == 7))
                        ht = h1p.tile([P, QC], f32, tag="h1t", name="h1t")
                        nc.scalar.activation(ht[:], pf[:], ACTF.Gelu,
                                             bias=bfc_sb[:, fi:fi + 1])
                        h1g.append(ht)
                    wm = []
                    for g in range(4):
                        w = wmpp.tile([P, C], f32, tag="wmp", name="wmp")
                        nc.sync.dma_start(out=w[:],
                                          in_=wmp_d[:, 4 * fig + g, :])
                        wm.append(w)
                    for qt in range(4):
                        for oc in range(2):
                            pm = psm.tile([P, 512], f32, name="psm")
                            for g in range(4):
                                nc.tensor.matmul(
                                    pm[:], h1g[g][:, P * qt:P * qt + P],
                                    wm[g][:, 512 * oc:512 * oc + 512],
                                    start=(g == 0), stop=(g == 3))
                            nc.vector.tensor_tensor(
                                out_acc[qt][:, 512 * oc:512 * oc + 512],
                                out_acc[qt][:, 512 * oc:512 * oc + 512],
                                pm[:], op=ALU.add)
                for qt in range(4):
                    nc.sync.dma_start(out=out_d[:, qt, :],
                                      in_=out_acc[qt][:])

    nc.compile()
    return nc


def _get_program():
    global _PROGRAM
    if _PROGRAM is None:
        _PROGRAM = _build_program()
    return _PROGRAM


def _rearr_pm(a, p=128):
    """[ (m p), n ] -> [p, m, n]"""
    m = a.shape[0] // p
    return np.ascontiguousarray(
        a.reshape(m, p, *a.shape[1:]).transpose(1, 0, *range(2, a.ndim + 1)))


def kernel(x, ln1_g, ln1_b, w_attn, b_attn, w_aproj, b_aproj,
           ln2_g, ln2_b, w_fc, b_fc, w_mproj, b_mproj, _trace=False):
    from concourse.bass_utils import run_bass_kernel_spmd

    x = np.asarray(x, np.float32)
    f64 = np.float64

    # fold LN affine transforms into the following matmul weights
    w_attn_e = (np.asarray(ln1_g, f64)[:, None] * np.asarray(w_attn, f64))
    b_attn_e = (np.asarray(b_attn, f64)
                + np.asarray(ln1_b, f64) @ np.asarray(w_attn, f64))
    w_fc_e = (np.asarray(ln2_g, f64)[:, None] * np.asarray(w_fc, f64))
    b_fc_e = (np.asarray(b_fc, f64)
              + np.asarray(ln2_b, f64) @ np.asarray(w_fc, f64))

    wqkv = _rearr_pm(w_attn_e.astype(np.float32))
    wap = _rearr_pm(np.asarray(w_aproj, np.float32))
    wfc = _rearr_pm(w_fc_e.astype(np.float32))
    wmp = _rearr_pm(np.asarray(w_mproj, np.float32))
    bqk = np.ascontiguousarray(
        b_attn_e[:2 * C].astype(np.float32).reshape(16, P).T)
    bv = np.ascontiguousarray(
        b_attn_e[2 * C:].astype(np.float32).reshape(8, P).T)
    bfc = np.ascontiguousarray(b_fc_e.astype(np.float32).reshape(32, P).T)
    bap = np.asarray(b_aproj, np.float32).reshape(1, C)
    bmp = np.asarray(b_mproj, np.float32).reshape(1, C)

    in_maps = []
    for c in range(N_CORES):
        b, Q = c // 4, c % 4
        xb = x[b]                       # [2048, 1024]
        xq = xb[QC * Q:QC * Q + QC]     # [512, 1024]
        in_maps.append({
            "xq": _rearr_pm(xq),
            "xk": _rearr_pm(xb),
            "qoff": np.full((P, 1), QC * Q, np.float32),
            "wqkv": wqkv, "waproj": wap, "wfc": wfc, "wmproj": wmp,
            "bqk_pm": bqk, "bv_pm": bv, "bfc_pm": bfc,
            "bap_row": bap, "bmp_row": bmp,
        })

    nc = _get_program()
    res = run_bass_kernel_spmd(nc, in_maps, list(range(N_CORES)),
                               trace=_trace)
    out = np.empty((2, T, C), np.float32)
    for c in range(N_CORES):
        b, Q = c // 4, c % 4
        o = res.results[c]["out"]       # [128, 4, 1024]
        out[b, QC * Q:QC * Q + QC] = o.transpose(1, 0, 2).reshape(QC, C)
    if _trace:
        return out, res
    return out
